# revision 17
# baseline (speedup 1.0000x reference)
"""GCN kernel v4: 8-band feature-pair-packed gather pipeline.

Per core: 12500 nodes degree-sorted into 8 bands (J<=1563, padded stride
1568). State s16 [20, 12544] fp16 (cols = band*1568 + j). Features packed in
PAIRS along the free axis for the gather path: packed table row r holds
(feat 2r, feat 2r+1) interleaved, so one gather index moves 2 features per
partition and a band needs only a 16-partition group (10 rows used) -- 8
bands gather in parallel per instruction.

Per layer: pack s16 -> s_pack [10, 25088] fp16 (2 strided DMAs) -> AllGather
-> T_cat [80, 25088] -> per block (16 = 8 shards x 2 halves, 6272 node-cols):
data tile [128, 12544] -> ONE ap_gather (d=2) slot-stream -> equal-S segment
reduces, job structure shared across bands and split by dst j-half, both
parities in one instruction -> per-half local_scatter back to j-order ->
DVE accumulate into pair-packed acc fp32 -> per band: accS fp16 staging,
parity-split PE matmuls (even/odd weight rows) + self-loop matmul ->
din/bias/relu/dout pointwise. Output unpermuted on host.
"""

import numpy as np

P = 128
N = 100000
E = 3200000
D = 20
NC = 8
Np = N // NC          # 12500
NL = 8                # bands per core
JB = 1568             # padded band stride (98*16)
HJ = 784              # j-half size
W = NL * JB           # 12544 state cols
NB = 16               # source blocks = 8 shards x 2 halves
TBLN = 6272           # node-cols per block (= W/2)
ZIDXN = 1563          # a always-zero (padded) node position in every block
L = 20                # conv layers


def _band_sizes():
    # 12500 = 4*1563 + 4*1562
    return [1563 if l < 4 else 1562 for l in range(NL)]


# ---------------------------------------------------------------------------
# Host-side planning (graph structure only)
# ---------------------------------------------------------------------------

def _plan(src, dst):
    src = np.asarray(src, dtype=np.int64)
    dst = np.asarray(dst, dtype=np.int64)
    out_deg = np.bincount(src, minlength=N) + 1
    in_deg = np.bincount(dst, minlength=N) + 1
    Js = _band_sizes()

    core_of = np.arange(N) // Np
    band = np.empty(N, np.int64)
    jpos = np.empty(N, np.int64)
    perm = [[None] * NL for _ in range(NC)]
    for c in range(NC):
        nodes = np.arange(c * Np, (c + 1) * Np)
        order = nodes[np.argsort(in_deg[nodes], kind="stable")]
        for l in range(NL):
            sel = order[l::NL]
            perm[c][l] = sel
            band[sel] = l
            jpos[sel] = np.arange(len(sel))
    scol = band * JB + jpos
    in_hi = (scol >= TBLN).astype(np.int64)
    src_blk = core_of * 2 + in_hi
    nodepos = scol - TBLN * in_hi          # 0..6271

    # dst cells keyed by (core, block, band, j-half, j-in-half)
    d_core = core_of[dst]
    l_d = band[dst]
    j_d = jpos[dst]
    h_d = j_d // HJ
    jh_d = j_d % HJ
    key = ((((d_core * NB + src_blk[src]) * NL + l_d) * 2 + h_d) * HJ + jh_d)
    eorder = np.argsort(key, kind="stable")
    elem_sorted = nodepos[src[eorder]].astype(np.int64)
    key_s = key[eorder]
    cells, cstart, ccount = np.unique(key_s, return_index=True,
                                      return_counts=True)
    cell_jh = cells % HJ
    cell_h = (cells // HJ) % 2
    cell_l = (cells // (2 * HJ)) % NL
    cell_b = (cells // (2 * HJ * NL)) % NB
    cell_c = cells // (2 * HJ * NL * NB)
    cell_group = cells // HJ               # (c,b,l,h) composite

    Smax = int(ccount.max())
    hist = np.zeros((NC, NB, NL, 2, Smax + 1), np.int64)
    np.add.at(hist, (cell_c, cell_b, cell_l, cell_h, ccount), 1)
    n_S_max = hist.max(axis=(0, 2))        # [NB, 2, Smax+1]

    jobs = {}          # b -> list of (h, S, T, cell_off, slot_off)
    M0 = np.zeros(NB, np.int64)            # cells in half 0
    M = np.zeros(NB, np.int64)
    len_b = np.zeros(NB, np.int64)
    for b in range(NB):
        co, so, jl = 0, 0, []
        for h in range(2):
            for S in range(1, Smax + 1):
                T = int(n_S_max[b, h, S])
                if T == 0:
                    continue
                jl.append((h, S, T, co, so))
                co += T
                so += T * S
            if h == 0:
                M0[b] = co
        jobs[b] = jl
        M[b] = co
        len_b[b] = so
    MP = int(M.max())
    L16_b = [int(-(-len_b[b] // 16)) * 16 for b in range(NB)]

    gidx = [[None] * NB for _ in range(NC)]
    lidx = [[None] * NB for _ in range(NC)]
    srt = np.lexsort((cell_jh, ccount, cell_group))
    group_key = cell_group[srt]
    gstart = np.searchsorted(group_key, np.arange(NC * NB * NL * 2 + 1))
    for c in range(NC):
        for b in range(NB):
            streams = np.full((NL, L16_b[b]), ZIDXN, np.int16)
            lix = np.full((NL, 2 * MP), -1, np.int16)
            for l in range(NL):
                for h in range(2):
                    gid = ((c * NB + b) * NL + l) * 2 + h
                    lo, hi = gstart[gid], gstart[gid + 1]
                    if hi <= lo:
                        continue
                    csel = srt[lo:hi]
                    cnts = ccount[csel]
                    jj = cell_jh[csel]
                    starts = cstart[csel]
                    for (hh, S, T, co_, so_) in jobs[b]:
                        if hh != h:
                            continue
                        p0 = int(np.searchsorted(cnts, S, "left"))
                        p1 = int(np.searchsorted(cnts, S, "right"))
                        k = p1 - p0
                        if k:
                            sl = slice(p0, p1)
                            ci = co_ + np.arange(k)
                            lix[l, 2 * ci] = 2 * jj[sl]
                            lix[l, 2 * ci + 1] = 2 * jj[sl] + 1
                            tgt = (so_ +
                                   np.repeat(np.arange(k) * S, S) +
                                   np.tile(np.arange(S), k))
                            srcpos = (np.repeat(starts[sl], S) +
                                      np.tile(np.arange(S), k))
                            streams[l, tgt] = elem_sorted[srcpos]
            garr = np.zeros((P, L16_b[b] // 16), np.int16)
            larr = np.full((P, 2 * MP), -1, np.int16)
            for l in range(NL):
                srow = streams[l].reshape(-1, 16).T
                garr[16 * l:16 * l + 16, :] = srow
                larr[16 * l:16 * l + 16, :] = lix[l]
            gidx[c][b] = garr
            lidx[c][b] = larr

    din = np.power(np.maximum(in_deg, 1), -0.5)
    dout = np.power(np.maximum(out_deg, 1), -0.5)
    din_fm = np.zeros((NC, 1, W), np.float16)
    dout_fm = np.zeros((NC, 1, W), np.float16)
    dd_fm = np.zeros((NC, 1, W), np.float16)
    dout_full = [[None] * NL for _ in range(NC)]
    din_full = [[None] * NL for _ in range(NC)]
    for c in range(NC):
        for l in range(NL):
            cols = slice(l * JB, l * JB + Js[l])
            din_fm[c, 0, cols] = din[perm[c][l]].astype(np.float16)
            dout_fm[c, 0, cols] = dout[perm[c][l]].astype(np.float16)
            dd_fm[c, 0, cols] = (din[perm[c][l]] *
                                 dout[perm[c][l]]).astype(np.float16)
            dout_full[c][l] = dout[perm[c][l]]
            din_full[c][l] = din[perm[c][l]]
    return dict(perm=perm, jobs=jobs, L16_b=L16_b, MP=MP, M0=M0, M=M,
                gidx=gidx, lidx=lidx, din_fm=din_fm, dout_fm=dout_fm,
                dd_fm=dd_fm, dout_full=dout_full, din_full=din_full)


# ---------------------------------------------------------------------------
# Device program
# ---------------------------------------------------------------------------

def build_program(plan, bias_zero=True, skip=()):
    import concourse.bacc as bacc
    import concourse.mybir as mybir
    import concourse.tile as tile

    f32 = mybir.dt.float32
    f16 = mybir.dt.float16
    i16 = mybir.dt.int16
    Alu = mybir.AluOpType
    Axis = mybir.AxisListType
    Act = mybir.ActivationFunctionType

    L16_b, MP, jobs = plan["L16_b"], plan["MP"], plan["jobs"]
    M0, M = plan["M0"], plan["M"]
    LMAX = max(L16_b)

    nc = bacc.Bacc("TRN2", target_bir_lowering=False, debug=False,
                   enable_asserts=False, num_devices=NC)

    t_s0 = nc.dram_tensor("s0_fm", [1, W], f16, kind="ExternalInput").ap()
    t_din = nc.dram_tensor("din_fm", [1, W], f16, kind="ExternalInput").ap()
    if bias_zero:
        # relu(din*x + 0)*dout == (din*dout)*relu(x): one resident table
        t_dd = nc.dram_tensor("dd_fm", [1, W], f16, kind="ExternalInput").ap()
    else:
        t_dout = nc.dram_tensor("dout_fm", [1, W], f16,
                                kind="ExternalInput").ap()
    t_w16 = nc.dram_tensor("w16", [D, D * L], f16, kind="ExternalInput").ap()
    t_wE = nc.dram_tensor("wE", [10, D * L], f16, kind="ExternalInput").ap()
    t_wO = nc.dram_tensor("wO", [10, D * L], f16, kind="ExternalInput").ap()
    t_selE = nc.dram_tensor("selE", [D, 10], f16,
                            kind="ExternalInput").ap()
    t_selO = nc.dram_tensor("selO", [D, 10], f16,
                            kind="ExternalInput").ap()
    t_bias = nc.dram_tensor("bias32", [D, L], f32, kind="ExternalInput").ap()
    goff = np.concatenate([[0], np.cumsum([L16_b[b] // 16
                                           for b in range(NB)])])
    t_gidx_all = nc.dram_tensor("gidx_all", [P, int(goff[-1])], i16,
                                kind="ExternalInput").ap()
    t_lidx_all = nc.dram_tensor("lidx_all", [P, NB * 2 * MP], i16,
                                kind="ExternalInput").ap()
    t_out = nc.dram_tensor("out_fm", [D, W], f16, kind="ExternalOutput").ap()

    rg = [list(range(NC))]

    with tile.TileContext(nc) as tc:
        with (
            tc.tile_pool(name="const", bufs=1) as const,
            tc.tile_pool(name="state", bufs=1) as statep,
            tc.tile_pool(name="datap", bufs=2 if bias_zero else 1) as datap,
            tc.tile_pool(name="streamp", bufs=2) as streamp,
            tc.tile_pool(name="partp", bufs=2) as partp,
            tc.tile_pool(name="lixp", bufs=2) as lixp,
            tc.tile_pool(name="pstp", bufs=2) as pstp,
            tc.tile_pool(name="accsp", bufs=1) as accsp,
            tc.tile_pool(name="packp", bufs=1) as packp,
            tc.tile_pool(name="hhp", bufs=2) as hhp,
            tc.tile_pool(name="ps", bufs=2, space="PSUM") as psp,
            tc.tile_pool(name="dram", bufs=1, space="DRAM") as dramp,
        ):
            if bias_zero:
                sb_dd16 = const.tile([D, W], f16, name="sb_dd16")
                nc.sync.dma_start(out=sb_dd16[0:1, :], in_=t_dd[:])
                for r in range(1, D):
                    nc.sync.dma_start(out=sb_dd16[r:r + 1, :],
                                      in_=sb_dd16[0:1, :])
            else:
                sb_din16 = const.tile([D, W], f16, name="sb_din16")
                nc.sync.dma_start(out=sb_din16[0:1, :], in_=t_din[:])
                sb_dout16 = const.tile([D, W], f16, name="sb_dout16")
                nc.sync.dma_start(out=sb_dout16[0:1, :], in_=t_dout[:])
                for r in range(1, D):
                    nc.sync.dma_start(out=sb_din16[r:r + 1, :],
                                      in_=sb_din16[0:1, :])
                    nc.sync.dma_start(out=sb_dout16[r:r + 1, :],
                                      in_=sb_dout16[0:1, :])
            sb_w16 = const.tile([D, D * L], f16, name="sb_w16")
            nc.sync.dma_start(out=sb_w16[:], in_=t_w16[:])
            sb_wE = const.tile([10, D * L], f16, name="sb_wE")
            nc.sync.dma_start(out=sb_wE[:], in_=t_wE[:])
            sb_wO = const.tile([10, D * L], f16, name="sb_wO")
            nc.sync.dma_start(out=sb_wO[:], in_=t_wO[:])
            sb_selE = const.tile([D, 10], f16, name="sb_selE")
            nc.sync.dma_start(out=sb_selE[:], in_=t_selE[:])
            sb_selO = const.tile([D, 10], f16, name="sb_selO")
            nc.sync.dma_start(out=sb_selO[:], in_=t_selO[:])
            sb_bias = const.tile([D, L], f32, name="sb_bias")
            nc.sync.dma_start(out=sb_bias[:], in_=t_bias[:])
            sb_gidx = []
            for b in range(NB):
                g = const.tile([P, L16_b[b] // 16], i16, name=f"sb_gi{b}")
                nc.sync.dma_start(
                    out=g[:],
                    in_=t_gidx_all[:, int(goff[b]):int(goff[b + 1])])
                sb_gidx.append(g)

            s16 = statep.tile([D, W], f16, name="s16")
            acc = statep.tile([P, 2 * JB], f32, name="acc")

            nc.vector.memset(s16[:], 0.0)
            nc.sync.dma_start(out=s16[0:1, :], in_=t_s0[:])

            for lay in range(1, L + 1):
                lc = slice((lay - 1) * D, lay * D)
                # ---- pack state into feature-pair table + AllGather ----
                s_pack = [dramp.tile([10, 2 * TBLN], f16,
                                     name=f"sp{lay}_{hf}", tag=f"sp{lay}_{hf}")
                          for hf in range(2)]
                QW = W // 4
                if "pack" not in skip:
                    # PE selector pack: psumE/O = parity-compressed feature
                    # rows; DVE interleaves into an SBUF staging quarter;
                    # one contiguous DMA per quarter to DRAM.
                    QW = W // 8
                    for q in range(8):
                        pq = packp.tile([10, 2 * QW], f16, name=f"pk{lay}_{q}",
                                        tag="pack")
                        for j0 in range(0, QW, 512):
                            n = min(512, QW - j0)
                            cols = slice(q * QW + j0, q * QW + j0 + n)
                            for pp, sel in ((0, sb_selE), (1, sb_selO)):
                                pm = psp.tile([D, 512], f32,
                                              name=f"pk{lay}_{q}_{j0}_{pp}",
                                              tag="psum")
                                nc.tensor.matmul(
                                    out=pm[0:10, 0:n], lhsT=sel[:],
                                    rhs=s16[:, cols], start=True, stop=True)
                                nc.vector.tensor_copy(
                                    out=pq[:, 2 * j0:2 * (j0 + n)]
                                    .rearrange("p (j two) -> p j two",
                                               two=2)[:, :, pp],
                                    in_=pm[0:10, 0:n])
                        qh, qr = q // 4, q % 4
                        nc.sync.dma_start(
                            out=s_pack[qh][:, 2 * qr * QW:2 * (qr + 1) * QW],
                            in_=pq[:])
                else:
                    nc.sync.dma_start(out=s_pack[0][:, 0:W // 2],
                                      in_=s16[0:10, 0:W // 2])
                T_half = [dramp.tile([NC * 10, 2 * TBLN], f16,
                                     name=f"T{lay}_{hf}", tag=f"T{lay}_{hf}",
                                     addr_space="Shared") for hf in range(2)]
                nc.gpsimd.collective_compute(
                    "AllGather", Alu.bypass, rg,
                    ins=[s_pack[0][:]], outs=[T_half[0][:]])

                def emit_ag1():
                    nc.gpsimd.collective_compute(
                        "AllGather", Alu.bypass, rg,
                        ins=[s_pack[1][:]], outs=[T_half[1][:]])

                if "ms" not in skip:
                    nc.vector.memset(acc[:], 0.0)

                # Software-pipelined block loop: emit apg(b) to the Pool
                # queue BEFORE lsc(b-1), so the Pool engine gathers block b
                # while the DVE reduces block b-1 (lsc waits on those).
                def gather_block(b):
                    cs, hh = b // 2, b % 2
                    lix = lixp.tile([P, 2 * MP], i16, name=f"li{lay}_{b}",
                                    tag="lix")
                    nc.sync.dma_start(
                        out=lix[:],
                        in_=t_lidx_all[:, b * 2 * MP:(b + 1) * 2 * MP])
                    data = datap.tile([P, 2 * TBLN], f16, name=f"d{lay}_{b}",
                                      tag="data")
                    for l in range(NL):
                        nc.sync.dma_start(
                            out=data[16 * l:16 * l + 10, :],
                            in_=T_half[hh][10 * cs:10 * cs + 10, :])
                    stream = streamp.tile([P, 2 * LMAX], f16,
                                          name=f"st{lay}_{b}", tag="stream")
                    if "apg" not in skip:
                        nc.gpsimd.ap_gather(
                            stream[:, 0:2 * L16_b[b]], data[:], sb_gidx[b][:],
                            channels=P, num_elems=TBLN, d=2,
                            num_idxs=L16_b[b])
                    return (b, stream, lix)

                def finish_block(st):
                    b, stream, lix = st
                    partial = partp.tile([P, 2 * MP], f16,
                                         name=f"pa{lay}_{b}", tag="partial")
                    with nc.allow_low_precision(reason="block partials f16"):
                        for (h, S, T, co, so) in (() if "red" in skip
                                                  else jobs[b]):
                            nc.vector.tensor_reduce(
                                out=partial[:, 2 * co:2 * (co + T)]
                                .rearrange("p (t two) -> p t two", two=2),
                                in_=stream[:, 2 * so:2 * (so + T * S)]
                                .rearrange("p (t s two) -> p t two s",
                                           two=2, s=S),
                                axis=Axis.X, op=Alu.add)
                    pstream = pstp.tile([P, 2 * JB], f16,
                                        name=f"pp{lay}_{b}", tag="pstream")
                    m0, m = int(M0[b]), int(M[b])
                    if "lsc" not in skip:
                        nc.gpsimd.local_scatter(
                            pstream[:, 0:JB], partial[:, 0:2 * m0],
                            lix[:, 0:2 * m0], channels=P, num_elems=JB,
                            num_idxs=2 * m0)
                        nc.gpsimd.local_scatter(
                            pstream[:, JB:2 * JB], partial[:, 2 * m0:2 * m],
                            lix[:, 2 * m0:2 * m], channels=P, num_elems=JB,
                            num_idxs=2 * (m - m0))
                    if "acc" not in skip:
                        nc.vector.tensor_tensor(
                            out=acc[:], in0=acc[:], in1=pstream[:],
                            op=Alu.add)

                pending = None
                order = list(range(0, NB, 2)) + list(range(1, NB, 2))
                for i, b in enumerate(order):
                    cur = gather_block(b)
                    if i == 0:
                        emit_ag1()
                    if pending is not None:
                        finish_block(pending)
                    pending = cur
                finish_block(pending)

                # ---- per band: staged matmuls + pointwise ----
                for l in (() if "mm" in skip else range(NL)):
                    accS = accsp.tile([10, 2 * JB], f16, name=f"as{lay}_{l}",
                                      tag="accS")
                    nc.gpsimd.dma_start(out=accS[:],
                                        in_=acc[16 * l:16 * l + 10, :])
                    for j0 in range(0, JB, 512):
                        n = min(512, JB - j0)
                        cols = slice(l * JB + j0, l * JB + j0 + n)
                        psum = psp.tile([D, 512], f32,
                                        name=f"ps{lay}_{l}_{j0}", tag="psum")
                        pair = accS[:, 2 * j0:2 * (j0 + n)].rearrange(
                            "p (j two) -> p j two", two=2)
                        nc.tensor.matmul(
                            out=psum[0:D, 0:n], lhsT=sb_wE[:, lc],
                            rhs=pair[:, :, 0], start=True, stop=False)
                        nc.tensor.matmul(
                            out=psum[0:D, 0:n], lhsT=sb_wO[:, lc],
                            rhs=pair[:, :, 1], start=False, stop=False)
                        nc.tensor.matmul(
                            out=psum[0:D, 0:n], lhsT=sb_w16[:, lc],
                            rhs=s16[:, cols], start=False, stop=True)
                        if bias_zero:
                            if lay < L:
                                # relu(din*x)*dout == dd*relu(x)
                                hh16 = hhp.tile([D, 512], f16,
                                                name=f"hh{lay}_{l}_{j0}",
                                                tag="hh")
                                nc.vector.tensor_copy(out=hh16[:, 0:n],
                                                      in_=psum[0:D, 0:n])
                                nc.scalar.activation(
                                    out=hh16[:, 0:n], in_=hh16[:, 0:n],
                                    func=Act.Relu, bias=0.0, scale=1.0)
                                nc.vector.tensor_tensor(
                                    out=s16[:, cols], in0=hh16[:, 0:n],
                                    in1=sb_dd16[:, cols], op=Alu.mult)
                            else:
                                # final din scale applied host-side in
                                # assemble() (pre-din value shipped out)
                                nc.vector.tensor_copy(out=s16[:, cols],
                                                      in_=psum[0:D, 0:n])
                            continue
                        hh16 = hhp.tile([D, 512], f16,
                                        name=f"hh{lay}_{l}_{j0}", tag="hh")
                        nc.vector.tensor_copy(out=hh16[:, 0:n],
                                              in_=psum[0:D, 0:n])
                        if True:
                            nc.vector.tensor_tensor(
                                out=hh16[:, 0:n], in0=hh16[:, 0:n],
                                in1=sb_din16[:, cols], op=Alu.mult)
                            if lay < L:
                                nc.scalar.activation(
                                    out=hh16[:, 0:n], in_=hh16[:, 0:n],
                                    func=Act.Relu,
                                    bias=sb_bias[:, lay - 1:lay], scale=1.0)
                                nc.vector.tensor_tensor(
                                    out=s16[:, cols], in0=hh16[:, 0:n],
                                    in1=sb_dout16[:, cols], op=Alu.mult)
                            else:
                                nc.vector.tensor_scalar(
                                    out=s16[:, cols], in0=hh16[:, 0:n],
                                    scalar1=sb_bias[:, lay - 1:lay],
                                    scalar2=None, op0=Alu.add)

            nc.sync.dma_start(out=t_out[:], in_=s16[:])

    nc.compile()
    return nc


def make_in_maps(inputs, plan):
    Js = _band_sizes()
    w_full = np.zeros((L, D, D), np.float64)
    w_full[0, 0:1, :] = np.asarray(inputs["W_start"], np.float64)
    for i in range(18):
        w_full[i + 1] = np.asarray(inputs["W_mid"][i], np.float64)
    w_full[L - 1] = np.asarray(inputs["W_final"], np.float64)
    w16 = np.zeros((D, D * L), np.float16)
    wE = np.zeros((10, D * L), np.float16)
    wO = np.zeros((10, D * L), np.float16)
    for i in range(L):
        w16[:, i * D:(i + 1) * D] = w_full[i].astype(np.float16)
        wE[:, i * D:(i + 1) * D] = w_full[i][0::2].astype(np.float16)
        wO[:, i * D:(i + 1) * D] = w_full[i][1::2].astype(np.float16)
    bias = np.zeros((D, L), np.float32)
    bias[:, 0] = np.asarray(inputs["b_start"], np.float32)
    for i in range(18):
        bias[:, i + 1] = np.asarray(inputs["b_mid"][i], np.float32)
    bias[:, L - 1] = np.asarray(inputs["b_final"], np.float32)

    feat = np.asarray(inputs["feat"], np.float64)[:, 0]
    in_maps = []
    for c in range(NC):
        s0 = np.zeros((1, W), np.float16)
        for l in range(NL):
            s0[0, l * JB:l * JB + Js[l]] = (feat[plan["perm"][c][l]] *
                                            plan["dout_full"][c][l]
                                            ).astype(np.float16)
        selE = np.zeros((D, 10), np.float16)
        selO = np.zeros((D, 10), np.float16)
        for r in range(10):
            selE[2 * r, r] = 1.0
            selO[2 * r + 1, r] = 1.0
        m = dict(s0_fm=s0,
                 din_fm=plan["din_fm"][c], dout_fm=plan["dout_fm"][c],
                 dd_fm=plan["dd_fm"][c],
                 w16=w16, wE=wE, wO=wO, selE=selE, selO=selO, bias32=bias)
        m["gidx_all"] = np.concatenate(plan["gidx"][c], axis=1)
        m["lidx_all"] = np.concatenate(plan["lidx"][c], axis=1)
        in_maps.append(m)
    return in_maps


def assemble(results, plan, bias_zero=True):
    Js = _band_sizes()
    out = np.zeros((N, D), np.float32)
    for c in range(NC):
        r = np.asarray(results[c]["out_fm"], np.float32)
        for l in range(NL):
            v = r[:, l * JB:l * JB + Js[l]].T
            if bias_zero:
                # device shipped the pre-din final value
                v = v * plan["din_full"][c][l][:, None]
            out[plan["perm"][c][l], :] = v
    return out[None]


_LAST = {}


def _bias_zero(inputs):
    return (not np.any(np.asarray(inputs["b_start"]))
            and not np.any(np.asarray(inputs["b_mid"]))
            and not np.any(np.asarray(inputs["b_final"])))


def run(inputs, trace=False):
    from concourse import bass_utils
    plan = _plan(inputs["src"], inputs["dst"])
    bz = _bias_zero(inputs)
    nc = build_program(plan, bias_zero=bz)
    in_maps = make_in_maps(inputs, plan)
    res = bass_utils.run_bass_kernel_spmd(
        nc, in_maps, core_ids=list(range(NC)), trace=trace)
    _LAST.update(nc=nc, in_maps=in_maps)
    return assemble(res.results, plan, bias_zero=bz), res


def _exec_state():
    """Persistent jitted executable + device-resident inputs for repeat
    timing (see kernel.py)."""
    if "sharded" in _LAST:
        return _LAST
    import jax
    from jax.sharding import Mesh, PartitionSpec, NamedSharding
    from jax.experimental.shard_map import shard_map
    from concourse import mybir
    from concourse.bass2jax import (_bass_exec_p, partition_id_tensor,
                                    install_neuronx_cc_hook)

    nc = _LAST["nc"]
    in_maps = _LAST["in_maps"]
    install_neuronx_cc_hook()
    pname = nc.partition_id_tensor.name if nc.partition_id_tensor else None
    in_names, out_names, out_avals, zero_shapes = [], [], [], []
    for alloc in nc.m.functions[0].allocations:
        if not isinstance(alloc, mybir.MemoryLocationSet):
            continue
        name = alloc.memorylocations[0].name
        if alloc.kind == "ExternalInput":
            if name != pname:
                in_names.append(name)
        elif alloc.kind == "ExternalOutput":
            out_names.append(name)
            shape = tuple(alloc.tensor_shape)
            dtype = mybir.dt.np(alloc.dtype)
            out_avals.append(jax.core.ShapedArray(shape, dtype))
            zero_shapes.append((shape, dtype))
    n_params = len(in_names)
    n_outs = len(out_avals)
    all_names = list(in_names) + out_names + ([pname] if pname else [])

    def _body(*args):
        operands = list(args)
        if pname is not None:
            operands.append(partition_id_tensor())
        outs = _bass_exec_p.bind(
            *operands, out_avals=tuple(out_avals), in_names=tuple(all_names),
            out_names=tuple(out_names), lowering_input_output_aliases=(),
            sim_require_finite=True, sim_require_nnan=True, nc=nc)
        return tuple(outs)

    devices = jax.devices()[:NC]
    mesh = Mesh(np.asarray(devices), ("core",))
    sharded = jax.jit(
        shard_map(_body, mesh=mesh,
                  in_specs=(PartitionSpec("core"),) * (n_params + n_outs),
                  out_specs=(PartitionSpec("core"),) * n_outs,
                  check_rep=False),
        donate_argnums=tuple(range(n_params, n_params + n_outs)),
        keep_unused=True)
    shardspec = NamedSharding(mesh, PartitionSpec("core"))
    concat_in = [np.concatenate([np.asarray(in_maps[c][nm])
                                 for c in range(NC)], axis=0)
                 for nm in in_names]
    dev_in = [jax.device_put(a, shardspec) for a in concat_in]
    jax.block_until_ready(dev_in)
    _LAST.update(sharded=sharded, dev_in=dev_in, zero_shapes=zero_shapes,
                 shardspec=shardspec, jax=jax)
    return _LAST


def run_again():
    """One full on-device execution with inputs already device-resident.
    Returns wall seconds for dispatch + execution (outputs left on device)."""
    import time
    st = _exec_state()
    jax = st["jax"]
    zeros = [jax.device_put(
        np.zeros((NC * s[0], *s[1:]), dt), st["shardspec"])
        for (s, dt) in st["zero_shapes"]]
    jax.block_until_ready(zeros)
    t0 = time.time()
    out_arrs = st["sharded"](*st["dev_in"], *zeros)
    jax.block_until_ready(out_arrs)
    return time.time() - t0


def kernel(**inputs):
    out, _ = run(inputs)
    return out.astype(np.float32)
